# revision 7
# baseline (speedup 1.0000x reference)
"""Single-head causal self-attention on 8 Trainium2 NeuronCores.

Problem: x[8, 4096, 1024], Wq/Wk/Wv[1024, 128] ->
  out[b] = softmax(causal((x[b] @ Wq) @ (x[b] @ Wk)^T / sqrt(128))) @ (x[b] @ Wv)

Sharding: data-parallel over batch -- each of the 8 cores handles one batch
element. Inputs are fed per-core as xT = x[b].T (layout prep on host) so the
contraction dim C lands on SBUF partitions.

Per-core kernel (T=4096, C=1024, HS=128), all matmuls in fp32r (full-rate
moving >= 256):
  Phase 1 (QKV): qT,kT [d=128, T] = sum_c Wq[c-chunk].T @ xT[c-chunk, :]
    vT likewise, then PE-transposed into v-natural [t, d] blocks.
  Phase 2 (attention), scores kept TRANSPOSED [kv, q] so that
    - PV needs no transposes: outT[d, q] += v_blk.T-free matmul
      (lhsT = v_blk [kv, d] natural, rhs = expT [kv, q]),
    - softmax denominator = partition-reduction done via a ones-vector matmul
      on a DVE-accumulated partial-sum tile.
    No max-subtraction: scaled scores are ~N(0,1), exp is safe in fp32.
    Causality: invalid 512-wide chunks skipped entirely; the diagonal
    128-block is masked with a precomputed upper-triangular 0/1 mask.
  Epilogue per q-group: PE-transpose outT -> out [q, d], scale rows by
    1/denominator, DMA out.
"""

import numpy as np

import concourse.bass as bass
import concourse.tile as tile
from concourse import bacc, mybir
from concourse.bass_utils import run_bass_kernel_spmd

B, T, C, HS = 8, 4096, 1024, 128
P = 128
NCORES = 8
CCH = C // P            # 8 c-chunks
NT = T // P             # 32 t/kv blocks of 128
TG = T // 512           # 8 t-groups of 512 (phase 1)
QG = T // 1024          # 4 q-groups of 1024 (phase 2)
SCALE = float(HS) ** -0.5

f32 = mybir.dt.float32
f32r = mybir.dt.float32r
EXP = mybir.ActivationFunctionType.Exp

_NC = None


def build_program():
    nc = bacc.Bacc()
    xT = nc.declare_dram_parameter("xT", [C, T], f32, isOutput=False)
    Wq = nc.declare_dram_parameter("Wq", [C, HS], f32, isOutput=False)
    Wk = nc.declare_dram_parameter("Wk", [C, HS], f32, isOutput=False)
    Wv = nc.declare_dram_parameter("Wv", [C, HS], f32, isOutput=False)
    # host-provided constants: [ones(2) | identity(128) | trimask(128) | zeros(512)]
    aux = nc.declare_dram_parameter("aux", [P, 770], f32, isOutput=False)
    out = nc.declare_dram_parameter("out", [T, HS], f32, isOutput=True)

    xT_r = xT[:].bitcast(f32r).rearrange("(j p) t -> p j t", p=P)
    w_views = [w[:].bitcast(f32r).rearrange("(j p) d -> p j d", p=P)
               for w in (Wq, Wk, Wv)]

    with tile.TileContext(nc) as tc:
        with (
            tc.tile_pool(name="consts", bufs=1) as consts,
            tc.tile_pool(name="big", bufs=1) as big,
        ):
            aux_sb = consts.tile([P, 770], f32r)
            nc.sync.dma_start(out=aux_sb[:], in_=aux[:].bitcast(f32r))
            ones = aux_sb[:, 0:2]
            ident = aux_sb[:, 2:130]
            trimask = aux_sb[:, 130:258]
            zeros = aux_sb[:, 258:770]

            w_sb = [consts.tile([P, CCH, HS], f32r, tag=f"w{i}", name=f"w{i}")
                    for i in range(3)]
            for w_t, w_v in zip(w_sb, w_views):
                nc.sync.dma_start(out=w_t[:], in_=w_v)

            qT = big.tile([P, T], f32r, tag="qT")   # [d, t]
            kT = big.tile([P, T], f32r, tag="kT")   # [d, t]
            vS = big.tile([P, NT, HS], f32r, tag="vS")  # [t-in-block, block, d]

            # ---------------- Phase 1: QKV projections ----------------
            with (
                tc.tile_pool(name="xin", bufs=2) as xin,
                tc.tile_pool(name="vtp", bufs=2) as vtp,
                tc.tile_pool(name="ps_qkv", bufs=2, space="PSUM") as ps_qkv,
                tc.tile_pool(name="ps_tr", bufs=2, space="PSUM") as ps_tr,
            ):
                for tg in range(TG):
                    t0 = 512 * tg
                    xt = xin.tile([P, CCH, 512], f32r)
                    nc.sync.dma_start(out=xt[:], in_=xT_r[:, :, t0:t0 + 512])

                    ps3 = [ps_qkv.tile([P, 512], f32, tag=f"ps{i}", name=f"ps{i}")
                           for i in range(3)]
                    for j in range(CCH):
                        for i in range(3):
                            nc.tensor.matmul(
                                ps3[i][:], lhsT=w_sb[i][:, j, :], rhs=xt[:, j, :],
                                start=(j == 0), stop=(j == CCH - 1),
                            )
                    nc.scalar.copy(qT[:, t0:t0 + 512], ps3[0][:])
                    nc.vector.tensor_copy(kT[:, t0:t0 + 512], ps3[1][:])
                    vt = vtp.tile([P, 512], f32r)
                    nc.vector.tensor_copy(vt[:], ps3[2][:])
                    for m in range(4):
                        tp = ps_tr.tile([P, P], f32r)
                        nc.tensor.transpose(tp[:], vt[:, 128 * m:128 * (m + 1)], ident)
                        eng = nc.scalar.copy if m % 2 == 0 else nc.vector.tensor_copy
                        eng(vS[:, 4 * tg + m, :], tp[:])

            # ---------------- Phase 2: causal attention ----------------
            with (
                tc.tile_pool(name="ptp", bufs=3) as ptp,
                tc.tile_pool(name="accp", bufs=2) as accp,
                tc.tile_pool(name="ocp", bufs=2) as ocp,
                tc.tile_pool(name="outsb", bufs=3) as outsb,
                tc.tile_pool(name="recipp", bufs=2) as recipp,
                tc.tile_pool(name="ps_s", bufs=2, space="PSUM") as ps_s,
                tc.tile_pool(name="ps_o", bufs=1, space="PSUM") as ps_o,
                tc.tile_pool(name="ps_t2", bufs=1, space="PSUM") as ps_t2,
                tc.tile_pool(name="ps_d", bufs=1, space="PSUM") as ps_d,
            ):
                for g in range(QG):
                    q0 = 1024 * g
                    o_ps = ps_o.tile([P, 1024], f32)
                    acc = accp.tile([P, 1024], f32r)
                    nkv = 8 * (g + 1)
                    for k in range(nkv):
                        vstart = max(0, 128 * k - q0)
                        s_ps = ps_s.tile([P, 1024], f32)
                        for c in range(2):
                            cq = 512 * c
                            if 128 * k >= q0 + cq + 512:
                                continue  # chunk fully above diagonal
                            nc.tensor.matmul(
                                s_ps[:, cq:cq + 512],
                                lhsT=kT[:, 128 * k:128 * (k + 1)],
                                rhs=qT[:, q0 + cq:q0 + cq + 512],
                                start=True, stop=True,
                            )
                        pt = ptp.tile([P, 1024], f32r)
                        nc.scalar.activation(
                            pt[:, vstart:1024], s_ps[:, vstart:1024], EXP, scale=SCALE)
                        if k >= 8 * g:  # diagonal block: mask kv > q
                            ms = vstart - (vstart % 512)
                            if vstart % 512:
                                nc.vector.tensor_copy(
                                    pt[:, ms:vstart], zeros[:, 0:vstart - ms])
                            nc.vector.tensor_mul(
                                pt[:, vstart:vstart + 128],
                                pt[:, vstart:vstart + 128], trimask)
                        if k == 0:
                            nc.vector.tensor_copy(acc[:], pt[:])
                        else:
                            nc.vector.tensor_add(
                                acc[:, vstart:1024], acc[:, vstart:1024],
                                pt[:, vstart:1024])
                        for c in range(2):
                            cq = 512 * c
                            if 128 * k >= q0 + cq + 512:
                                continue
                            last_k = 8 * g + 4 * c + 3
                            nc.tensor.matmul(
                                o_ps[:, cq:cq + 512],
                                lhsT=vS[:, k, :], rhs=pt[:, cq:cq + 512],
                                start=(k == 0), stop=(k == last_k),
                            )

                    # epilogue: denominators, transpose, normalize, store
                    d_ps = ps_d.tile([P, 8, 2], f32)
                    for m in range(8):
                        nc.tensor.matmul(
                            d_ps[:, m, :],
                            lhsT=acc[:, 128 * m:128 * (m + 1)], rhs=ones,
                            start=True, stop=True,
                        )
                    recip = recipp.tile([P, 8], f32)
                    nc.vector.reciprocal(recip[:], d_ps[:, :, 0])

                    oc = ocp.tile([P, 1024], f32r)
                    nc.scalar.copy(oc[:], o_ps[:])
                    for m in range(8):
                        tr = ps_t2.tile([P, P], f32r)
                        nc.tensor.transpose(
                            tr[:], oc[:, 128 * m:128 * (m + 1)], ident)
                        osb = outsb.tile([P, HS], f32)
                        nc.vector.tensor_scalar_mul(osb[:], tr[:], recip[:, m:m + 1])
                        nc.sync.dma_start(
                            out=out[q0 + 128 * m:q0 + 128 * (m + 1), :], in_=osb[:])

    nc.finalize()
    return nc


def _get_nc():
    global _NC
    if _NC is None:
        _NC = build_program()
    return _NC


def kernel(x, Wq, Wk, Wv):
    assert x.shape == (B, T, C) and Wq.shape == (C, HS)
    nc = _get_nc()
    x = np.asarray(x, dtype=np.float32)
    aux = np.zeros((P, 770), dtype=np.float32)
    aux[:, 0:2] = 1.0
    aux[:, 2:130] = np.eye(P, dtype=np.float32)
    iu = np.triu(np.ones((P, P), dtype=np.float32))  # 1 where kv <= q
    aux[:, 130:258] = iu
    in_maps = [
        {
            "xT": np.ascontiguousarray(x[b].T),
            "Wq": np.asarray(Wq, dtype=np.float32),
            "Wk": np.asarray(Wk, dtype=np.float32),
            "Wv": np.asarray(Wv, dtype=np.float32),
            "aux": aux,
        }
        for b in range(NCORES)
    ]
    res = run_bass_kernel_spmd(nc, in_maps, list(range(NCORES)))
    return np.stack([res.results[b]["out"] for b in range(NCORES)])


# revision 8
# speedup vs baseline: 1.0755x; 1.0755x over previous
"""Single-head causal self-attention on 8 Trainium2 NeuronCores.

Problem: x[8, 4096, 1024], Wq/Wk/Wv[1024, 128] ->
  out[b] = softmax(causal((x[b] @ Wq) @ (x[b] @ Wk)^T / sqrt(128))) @ (x[b] @ Wv)

Sharding: data-parallel over batch -- each of the 8 cores handles one batch
element. Inputs are fed per-core as xT = x[b].T (layout prep on host) so the
contraction dim C lands on SBUF partitions.

Per-core kernel (T=4096, C=1024, HS=128), all matmuls in fp32r (full-rate
moving >= 256):
  Phase 1 (QKV): qT,kT [d=128, T] = sum_c Wq[c-chunk].T @ xT[c-chunk, :]
    vT likewise, then PE-transposed into v-natural [t, d] blocks.
  Phase 2 (attention), scores kept TRANSPOSED [kv, q] so that
    - PV needs no transposes: outT[d, q] += v_blk.T-free matmul
      (lhsT = v_blk [kv, d] natural, rhs = expT [kv, q]),
    - softmax denominator = partition-reduction done via a ones-vector matmul
      on a DVE-accumulated partial-sum tile.
    No max-subtraction: scaled scores are ~N(0,1), exp is safe in fp32.
    Causality: invalid 512-wide chunks skipped entirely; the diagonal
    128-block is masked with a precomputed upper-triangular 0/1 mask.
  Epilogue per q-group: PE-transpose outT -> out [q, d], scale rows by
    1/denominator, DMA out.
"""

import numpy as np

import concourse.bass as bass
import concourse.tile as tile
from concourse import bacc, mybir
from concourse.bass_utils import run_bass_kernel_spmd

B, T, C, HS = 8, 4096, 1024, 128
P = 128
NCORES = 8
CCH = C // P            # 8 c-chunks
NT = T // P             # 32 t/kv blocks of 128
TG = T // 512           # 8 t-groups of 512 (phase 1)
QG = T // 1024          # 4 q-groups of 1024 (phase 2)
SCALE = float(HS) ** -0.5

f32 = mybir.dt.float32
f32r = mybir.dt.float32r
bf16 = mybir.dt.bfloat16
EXP = mybir.ActivationFunctionType.Exp

_NC = None


def build_program():
    nc = bacc.Bacc()
    xT = nc.declare_dram_parameter("xT", [C, T], f32, isOutput=False)
    Wq = nc.declare_dram_parameter("Wq", [C, HS], f32, isOutput=False)
    Wk = nc.declare_dram_parameter("Wk", [C, HS], f32, isOutput=False)
    Wv = nc.declare_dram_parameter("Wv", [C, HS], f32, isOutput=False)
    # host-provided constants: [ones(2) | identity(128) | trimask(128) | zeros(512)]
    aux = nc.declare_dram_parameter("aux", [P, 770], f32, isOutput=False)
    out = nc.declare_dram_parameter("out", [T, HS], f32, isOutput=True)

    xT_r = xT[:].bitcast(f32r).rearrange("(j p) t -> p j t", p=P)
    w_views = [w[:].bitcast(f32r).rearrange("(j p) d -> p j d", p=P)
               for w in (Wq, Wk, Wv)]

    with tile.TileContext(nc) as tc:
        with (
            tc.tile_pool(name="consts", bufs=1) as consts,
            tc.tile_pool(name="big", bufs=1) as big,
        ):
            aux_sb = consts.tile([P, 770], f32r)
            nc.sync.dma_start(out=aux_sb[:], in_=aux[:].bitcast(f32r))
            ones = aux_sb[:, 0:2]
            ident = aux_sb[:, 2:130]
            trimask = aux_sb[:, 130:258]
            zeros = aux_sb[:, 258:770]

            w_sb = [consts.tile([P, CCH, HS], f32r, tag=f"w{i}", name=f"w{i}")
                    for i in range(3)]
            for w_t, w_v in zip(w_sb, w_views):
                nc.sync.dma_start(out=w_t[:], in_=w_v)

            qT = big.tile([P, T], f32r, tag="qT")   # [d, t]
            kT = big.tile([P, T], f32r, tag="kT")   # [d, t]
            vS = big.tile([P, NT, HS], bf16, tag="vS")  # [t-in-block, block, d]

            # ---------------- Phase 1: QKV projections ----------------
            with (
                tc.tile_pool(name="xin", bufs=2) as xin,
                tc.tile_pool(name="vtp", bufs=2) as vtp,
                tc.tile_pool(name="ps_qkv", bufs=2, space="PSUM") as ps_qkv,
                tc.tile_pool(name="ps_tr", bufs=2, space="PSUM") as ps_tr,
            ):
                for tg in range(TG):
                    t0 = 512 * tg
                    xts = [xin.tile([P, 512], f32r, tag=f"xt{j}", name=f"xt{j}")
                           for j in range(CCH)]
                    for j in range(CCH):
                        nc.sync.dma_start(out=xts[j][:], in_=xT_r[:, j, t0:t0 + 512])

                    ps3 = [ps_qkv.tile([P, 512], f32, tag=f"ps{i}", name=f"ps{i}")
                           for i in range(3)]
                    for j in range(CCH):
                        for i in range(3):
                            nc.tensor.matmul(
                                ps3[i][:], lhsT=w_sb[i][:, j, :], rhs=xts[j][:],
                                start=(j == 0), stop=(j == CCH - 1),
                            )
                    nc.scalar.copy(qT[:, t0:t0 + 512], ps3[0][:])
                    nc.vector.tensor_copy(kT[:, t0:t0 + 512], ps3[1][:])
                    vt = vtp.tile([P, 512], f32r)
                    nc.vector.tensor_copy(vt[:], ps3[2][:])
                    for m in range(4):
                        tp = ps_tr.tile([P, P], f32r)
                        nc.tensor.transpose(tp[:], vt[:, 128 * m:128 * (m + 1)], ident)
                        eng = nc.scalar.copy if m % 2 == 0 else nc.vector.tensor_copy
                        eng(vS[:, 4 * tg + m, :], tp[:])

            # ---------------- Phase 2: causal attention ----------------
            with (
                tc.tile_pool(name="ptp", bufs=3) as ptp,
                tc.tile_pool(name="accp", bufs=2) as accp,
                tc.tile_pool(name="ocp", bufs=2) as ocp,
                tc.tile_pool(name="outsb", bufs=3) as outsb,
                tc.tile_pool(name="recipp", bufs=2) as recipp,
                tc.tile_pool(name="ps_s", bufs=2, space="PSUM") as ps_s,
                tc.tile_pool(name="ps_o", bufs=1, space="PSUM") as ps_o,
                tc.tile_pool(name="ps_t2", bufs=1, space="PSUM") as ps_t2,
                tc.tile_pool(name="ps_d", bufs=1, space="PSUM") as ps_d,
            ):
                for g in range(QG):
                    q0 = 1024 * g
                    o_ps = ps_o.tile([P, 1024], f32)
                    acc = accp.tile([P, 1024], f32r)
                    nkv = 8 * (g + 1)
                    for k in range(nkv):
                        vstart = max(0, 128 * k - q0)
                        s_ps = ps_s.tile([P, 1024], f32)
                        for c in range(2):
                            cq = 512 * c
                            lc = max(0, vstart - cq)
                            if lc >= 512:
                                continue  # chunk fully above diagonal
                            nc.tensor.matmul(
                                s_ps[:, cq + lc:cq + 512],
                                lhsT=kT[:, 128 * k:128 * (k + 1)],
                                rhs=qT[:, q0 + cq + lc:q0 + cq + 512],
                                start=True, stop=True,
                            )
                        pt = ptp.tile([P, 1024], bf16)
                        nc.scalar.activation(
                            pt[:, vstart:1024], s_ps[:, vstart:1024], EXP, scale=SCALE)
                        if k >= 8 * g:  # diagonal block: mask kv > q
                            ms = vstart - (vstart % 512)
                            if vstart % 512:
                                nc.vector.tensor_copy(
                                    pt[:, ms:vstart], zeros[:, 0:vstart - ms])
                            nc.vector.tensor_mul(
                                pt[:, vstart:vstart + 128],
                                pt[:, vstart:vstart + 128], trimask)
                        if k == 0:
                            nc.vector.tensor_copy(acc[:], pt[:])
                        else:
                            nc.vector.tensor_add(
                                acc[:, vstart:1024], acc[:, vstart:1024],
                                pt[:, vstart:1024])
                        for c in range(2):
                            cq = 512 * c
                            lc = max(0, vstart - cq)
                            if lc >= 512:
                                continue
                            last_k = 8 * g + 4 * c + 3
                            nc.tensor.matmul(
                                o_ps[:, cq + lc:cq + 512],
                                lhsT=vS[:, k, :], rhs=pt[:, cq + lc:cq + 512],
                                start=(k == 0), stop=(k == last_k),
                            )

                    # epilogue: denominators, transpose, normalize, store
                    d_ps = ps_d.tile([P, 8, 2], f32)
                    for m in range(8):
                        nc.tensor.matmul(
                            d_ps[:, m, :],
                            lhsT=acc[:, 128 * m:128 * (m + 1)], rhs=ones,
                            start=True, stop=True,
                        )
                    recip = recipp.tile([P, 8], f32)
                    nc.vector.reciprocal(recip[:], d_ps[:, :, 0])

                    oc = ocp.tile([P, 1024], f32r)
                    nc.scalar.copy(oc[:], o_ps[:])
                    for m in range(8):
                        tr = ps_t2.tile([P, P], f32r)
                        nc.tensor.transpose(
                            tr[:], oc[:, 128 * m:128 * (m + 1)], ident)
                        osb = outsb.tile([P, HS], f32)
                        nc.vector.tensor_scalar_mul(osb[:], tr[:], recip[:, m:m + 1])
                        nc.sync.dma_start(
                            out=out[q0 + 128 * m:q0 + 128 * (m + 1), :], in_=osb[:])

    nc.finalize()
    return nc


def _get_nc():
    global _NC
    if _NC is None:
        _NC = build_program()
    return _NC


def kernel(x, Wq, Wk, Wv):
    assert x.shape == (B, T, C) and Wq.shape == (C, HS)
    nc = _get_nc()
    x = np.asarray(x, dtype=np.float32)
    aux = np.zeros((P, 770), dtype=np.float32)
    aux[:, 0:2] = 1.0
    aux[:, 2:130] = np.eye(P, dtype=np.float32)
    iu = np.triu(np.ones((P, P), dtype=np.float32))  # 1 where kv <= q
    aux[:, 130:258] = iu
    in_maps = [
        {
            "xT": np.ascontiguousarray(x[b].T),
            "Wq": np.asarray(Wq, dtype=np.float32),
            "Wk": np.asarray(Wk, dtype=np.float32),
            "Wv": np.asarray(Wv, dtype=np.float32),
            "aux": aux,
        }
        for b in range(NCORES)
    ]
    res = run_bass_kernel_spmd(nc, in_maps, list(range(NCORES)))
    return np.stack([res.results[b]["out"] for b in range(NCORES)])
